# revision 3
# baseline (speedup 1.0000x reference)
"""Trainium2 Bass kernel for nn_HeatmapEncoder.

Math per (b, s, c) and per coordinate set (gaze, hand):
    g = exp(-((gx-cx)^2 + (gy-cy)^2) / (2 sigma^2))   on a 336x336 grid
    g = g / (sum(g) + eps)            (zeroed when cx+cy <= 0)
    unified = g_gaze + g_hand
    out = unified / (max(unified) + eps)

The Gaussian is separable, so each unified map is rank-2.  Each map is
generated ONCE by three K=6 bf16 matmuls (hi/lo split of each fp32
factor; the yl*xl term is dropped, rel err ~2^-16):
    rows (per set): (yh, xh), (yh, xl), (yl, xh)
Sum-normalization is folded into the y factors.

The peak is computed ANALYTICALLY before any map exists: for two
equal-sigma isotropic Gaussians the continuous max lies on the segment
between the two centers, so sampling
    f(t) = A exp(-D^2 t^2 / 2s^2) + B exp(-D^2 (1-t)^2 / 2s^2)
at SEG points (A, B = per-set amplitudes, D^2 = |c1-c2|^2 from the
host) bounds the grid peak to ~0.3%.  Gaze rows sample t^2, hand rows
(1-t)^2; a [64,32] pairing matmul adds the two sets per map, and a
[32,32] identity matmul broadcasts 1/(peak+eps) across partitions —
no cross-partition DMA.  Each map then needs a single fused
scale-drain (PSUM -> SBUF tensor_scalar_mul, DVE/ACT alternating)
followed by its output DMA (sync/gpsimd queues alternating), so the
output bus runs at full 16-engine rate (~360 GB/s) instead of being
starved by the per-map copy+max+all_reduce+scale chain.

Layout: map j = 4*b + q keeps its 6 factor rows at SBUF partitions
32*q .. 32*q+5, free block b (PE row-tiles are tied to 32-aligned
partition groups; cycling q hides LDWEIGHTS under matmuls).  Map rows
are interleaved y = 3*p + c so each map is a single contiguous DRAM
range for the output DMA.

Sharding: pure data parallel over batch B=8 across the 8 cores.
"""

import functools
from contextlib import ExitStack

import numpy as np
import ml_dtypes

try:
    import concourse.bass as bass
except ImportError:  # pragma: no cover
    import sys

    sys.path.insert(0, "/opt/trn_rl_repo")
    import concourse.bass as bass

import concourse.tile as tile
from concourse import bacc, mybir
from concourse.bass_utils import run_bass_kernel_spmd

H = W = 336
P = 112  # partitions per y-chunk; y = 3*p + c  (c in 0..2)
NCH = 3
S_DIM, C_DIM = 8, 4
NMAPS = S_DIM * C_DIM  # 32 maps per core
NR = 2 * NMAPS  # 64 factor rows (map-major, gaze/hand interleaved)
NB = 8  # free blocks in the aligned factor layout (map j = 4*b + q)
N_CORES = 8
SIGMA = 10.0 / 336.0
EXP_SCALE = -1.0 / (2.0 * SIGMA * SIGMA)
EPS = 1e-6
SEG = 512  # segment samples for the analytic peak

F32 = mybir.dt.float32
BF16 = mybir.dt.bfloat16
AF = mybir.ActivationFunctionType
ALU = mybir.AluOpType
AX = mybir.AxisListType


def _emit(nc, tc, ctx, negc_in, dsq_in, out_t, grid_const, ts_const,
          pair_const, eye_const, ystg, xstg):
    const = ctx.enter_context(tc.tile_pool(name="const", bufs=1))
    fact = ctx.enter_context(tc.tile_pool(name="fact", bufs=1))
    ffac = ctx.enter_context(tc.tile_pool(name="ffac", bufs=1))
    small = ctx.enter_context(tc.tile_pool(name="small", bufs=2))
    sstage = ctx.enter_context(tc.tile_pool(name="sstage", bufs=16))
    pmap = ctx.enter_context(tc.tile_pool(name="pmap", bufs=2, space="PSUM"))
    ppk = ctx.enter_context(tc.tile_pool(name="ppk", bufs=1, space="PSUM"))

    # ---- early ACT table preload via dummy exp on a memset tile ----
    dum = small.tile([1, 16], F32, tag="dum")
    nc.gpsimd.memset(dum[:], 0.0)
    dum2 = small.tile([1, 16], F32, tag="dum2")
    nc.scalar.activation(dum2[:], dum[:], AF.Exp, bias=0.0, scale=1.0)
    ONES = const.tile([NMAPS, P], BF16)
    nc.gpsimd.memset(ONES[:], 1.0)

    # ---- constants / inputs ----
    # sync: G, NC2 gate the factor chain; TS/DSQ feed the peak chain
    G = const.tile([NR, W], F32)
    nc.sync.dma_start(G[:], grid_const.ap())
    NC2 = const.tile([NR, 2], F32)
    nc.sync.dma_start(NC2[:], negc_in.ap())
    TS = const.tile([NR, SEG], F32)
    nc.sync.dma_start(TS[:], ts_const.ap())
    DSQ = const.tile([NR, 1], F32)
    nc.sync.dma_start(DSQ[:], dsq_in.ap())
    PAIR = const.tile([NR, NMAPS], BF16)
    nc.gpsimd.dma_start(PAIR[:], pair_const.ap())
    EYE = const.tile([NMAPS, NMAPS], BF16)
    nc.gpsimd.dma_start(EYE[:], eye_const.ap())

    # ---- 1-D gaussian factors, dense [64, 336] fp32 (x side first:
    # the x factors are unscaled and gate the scatters) ----
    sqx = fact.tile([NR, W], F32)
    nc.scalar.activation(sqx[:], G[:], AF.Square, bias=NC2[:, 0:1], scale=1.0)
    fxv = fact.tile([NR, W], F32)
    nc.scalar.activation(fxv[:], sqx[:], AF.Exp, bias=0.0, scale=EXP_SCALE)
    sqy = fact.tile([NR, W], F32)
    nc.scalar.activation(sqy[:], G[:], AF.Square, bias=NC2[:, 1:2], scale=1.0)
    fyv = fact.tile([NR, W], F32)
    nc.scalar.activation(fyv[:], sqy[:], AF.Exp, bias=0.0, scale=EXP_SCALE)

    # x-side hi/lo split (UNSCALED); bounce to DRAM early on gpsimd queue
    xh = fact.tile([NR, W], BF16)
    nc.vector.tensor_copy(xh[:], fxv[:])
    xl = fact.tile([NR, W], BF16)
    nc.vector.tensor_sub(xl[:], fxv[:], xh[:])
    nc.gpsimd.dma_start(xstg.ap()[0], xh[:])
    nc.gpsimd.dma_start(xstg.ap()[1], xl[:])
    nc.gpsimd.dma_start(xstg.ap()[2], xh[:])

    # normalization scale a = valid / (Sx*Sy + eps) folded into y factors
    sx = small.tile([NR, 1], F32, tag="sx")
    nc.vector.reduce_sum(sx[:], fxv[:], axis=AX.X)
    sy = small.tile([NR, 1], F32, tag="sy")
    nc.vector.reduce_sum(sy[:], fyv[:], axis=AX.X)
    ss = small.tile([NR, 1], F32, tag="ss")
    nc.vector.tensor_mul(ss[:], sx[:], sy[:])
    sse = small.tile([NR, 1], F32, tag="sse")
    nc.vector.tensor_scalar_add(sse[:], ss[:], EPS)
    rec = small.tile([NR, 1], F32, tag="rec")
    nc.vector.reciprocal(rec[:], sse[:])
    vs = small.tile([NR, 1], F32, tag="vs")
    nc.vector.tensor_add(vs[:], NC2[:, 0:1], NC2[:, 1:2])
    vm = small.tile([NR, 1], F32, tag="vm")  # valid: (-cx)+(-cy) < 0
    nc.vector.tensor_scalar(vm[:], vs[:], 0.0, None, op0=ALU.is_lt)
    av = small.tile([NR, 1], F32, tag="av")
    nc.vector.tensor_mul(av[:], rec[:], vm[:])
    fys = fact.tile([NR, W], F32)
    nc.vector.tensor_scalar_mul(fys[:], fyv[:], av[:, 0:1])

    # y-side hi/lo split (carries the a-scale); bounce on sync queue
    yh = fact.tile([NR, W], BF16)
    nc.vector.tensor_copy(yh[:], fys[:])
    yl = fact.tile([NR, W], BF16)
    nc.vector.tensor_sub(yl[:], fys[:], yh[:])
    nc.sync.dma_start(ystg.ap()[0], yh[:])
    nc.sync.dma_start(ystg.ap()[1], yh[:])
    nc.sync.dma_start(ystg.ap()[2], yl[:])

    # ---- analytic peak chain (no map read-back, no DRAM hop) ----
    # f_row(t_i) = av_row * exp(EXP_SCALE * D^2 * {t^2 | (1-t)^2})
    us = fact.tile([NR, SEG], F32)
    nc.vector.tensor_scalar_mul(us[:], TS[:], DSQ[:, 0:1])
    ee = fact.tile([NR, SEG], F32)
    nc.scalar.activation(ee[:], us[:], AF.Exp, bias=0.0, scale=EXP_SCALE)
    pp = fact.tile([NR, SEG], BF16)
    nc.vector.tensor_scalar_mul(pp[:], ee[:], av[:, 0:1])
    # pair matmul: psum[j, i] = f_gaze(2j) + f_hand(2j+1) = unified on segment
    pk_ps = ppk.tile([NMAPS, SEG], F32, tag="pkps")
    nc.tensor.matmul(pk_ps[:], PAIR[:], pp[:], start=True, stop=True,
                     tile_position=(0, 0))
    pk = small.tile([NMAPS, 1], F32, tag="pk")
    nc.vector.reduce_max(pk[:], pk_ps[:], axis=AX.X)
    pke = small.tile([NMAPS, 1], F32, tag="pke")
    nc.vector.tensor_scalar_add(pke[:], pk[:], EPS)
    # broadcast pke across partitions via identity matmul, then 1/x
    pkb = small.tile([NMAPS, P], BF16, tag="pkb")
    nc.vector.tensor_scalar_mul(pkb[:], ONES[:], pke[:, 0:1])
    rg_ps = ppk.tile([P, NMAPS], F32, tag="rgps")
    nc.tensor.matmul(rg_ps[:], pkb[:], EYE[:], start=True, stop=True,
                     tile_position=(0, 0))
    RG = const.tile([P, NMAPS], F32)
    nc.vector.reciprocal(RG[:], rg_ps[:])

    # ---- scatter the bounced factors into the 32-aligned 6-row layout ----
    # one DMA per (q, side): dest rows 32q+3t+u <- stg[u, 8b+2q+t, x]
    FYq = [ffac.tile([128, NB, W], BF16, name=f"FY{q}", tag=f"fy{q}")
           for q in range(4)]
    FXq = [ffac.tile([128, NB, W], BF16, name=f"FX{q}", tag=f"fx{q}")
           for q in range(4)]
    ya = ystg.ap().rearrange("u (b g) x -> g u b x", g=NB)
    xa = xstg.ap().rearrange("u (b g) x -> g u b x", g=NB)
    for q in range(4):
        nc.sync.dma_start(FYq[q][32 * q:32 * q + 6, :, :], ya[2 * q:2 * q + 2])
        nc.gpsimd.dma_start(FXq[q][32 * q:32 * q + 6, :, :],
                            xa[2 * q:2 * q + 2])

    # DRAM view matching stage layout: out[m, y, x], y = 3p+c, z = 336c+x
    dview = out_t.ap().rearrange("m (p c) x -> p m (c x)", p=P)

    def map_matmuls(j, pt):
        q, b = j % 4, j // 4
        rhs = FXq[q][32 * q:32 * q + 6, b, :]
        for cix in range(NCH):
            lhsT = FYq[q][32 * q:32 * q + 6, b, cix::3]
            nc.tensor.matmul(pt[:, cix * 512:cix * 512 + W], lhsT, rhs,
                             start=True, stop=True, tile_position=(32 * q, 0))

    for j in range(NMAPS):
        pt = pmap.tile([P, NCH * 512], F32, tag="pmap")
        map_matmuls(j, pt)
        pview = pt[:].rearrange("p (c z) -> p c z", c=NCH)[:, :, 0:W]
        st = sstage.tile([P, NCH * W], F32, tag="sst")
        sview = st[:].rearrange("p (c x) -> p c x", c=NCH)
        # fused scale-drain is the SOLE psum reader (frees the slot)
        if j % 2 == 0:
            nc.vector.tensor_scalar_mul(sview, pview, RG[:, j:j + 1])
            nc.sync.dma_start(dview[:, j:j + 1, :], st[:])
        else:
            nc.scalar.mul(sview, pview, RG[:, j:j + 1])
            nc.gpsimd.dma_start(dview[:, j:j + 1, :], st[:])


@functools.lru_cache(maxsize=1)
def _build():
    nc = bacc.Bacc("TRN2", target_bir_lowering=False, debug=False)
    negc_in = nc.dram_tensor("negc", [NR, 2], F32, kind="ExternalInput")
    dsq_in = nc.dram_tensor("dsq", [NR, 1], F32, kind="ExternalInput")
    out_t = nc.dram_tensor("out", [NMAPS, H, W], F32, kind="ExternalOutput")

    grid = (np.arange(W, dtype=np.float64) / (W - 1)).astype(np.float32)
    grid_const = nc.inline_tensor(np.tile(grid, (NR, 1)), name="gridc")

    t = np.arange(SEG, dtype=np.float64) / (SEG - 1)
    ts = np.empty((NR, SEG), dtype=np.float32)
    ts[0::2] = (t ** 2).astype(np.float32)
    ts[1::2] = ((1.0 - t) ** 2).astype(np.float32)
    ts_const = nc.inline_tensor(ts, name="tsc")

    pair = np.zeros((NR, NMAPS), dtype=ml_dtypes.bfloat16)
    pair[np.arange(NR), np.arange(NR) // 2] = 1
    pair_const = nc.inline_tensor(pair, name="pairc")
    eye_const = nc.inline_tensor(np.eye(NMAPS, dtype=ml_dtypes.bfloat16),
                                 name="eyec")

    ystg = nc.dram_tensor("ystg", [3, NR, W], BF16)
    xstg = nc.dram_tensor("xstg", [3, NR, W], BF16)

    with tile.TileContext(nc) as tc, ExitStack() as ctx:
        _emit(nc, tc, ctx, negc_in, dsq_in, out_t, grid_const, ts_const,
              pair_const, eye_const, ystg, xstg)
    nc.compile()
    return nc


def _in_map_for(gaze, hand, b):
    cg = np.asarray(gaze[b], dtype=np.float32).reshape(NMAPS, 2)
    ch = np.asarray(hand[b], dtype=np.float32).reshape(NMAPS, 2)
    inter = np.stack([cg, ch], axis=1).reshape(NR, 2)  # row 2*j + t
    d2 = ((cg - ch) ** 2).sum(-1)  # |c_gaze - c_hand|^2 per map
    dsq = np.repeat(d2, 2)[:, None].astype(np.float32)
    return {"negc": np.ascontiguousarray(-inter),
            "dsq": np.ascontiguousarray(dsq)}


def kernel(gaze_coords, hand_coords, _trace=False, **trace_kwargs):
    gaze_coords = np.asarray(gaze_coords, dtype=np.float32)
    hand_coords = np.asarray(hand_coords, dtype=np.float32)
    B = gaze_coords.shape[0]
    assert B == N_CORES, f"expected batch {N_CORES}, got {B}"
    nc = _build()
    in_maps = [_in_map_for(gaze_coords, hand_coords, b) for b in range(B)]
    res = run_bass_kernel_spmd(nc, in_maps, list(range(N_CORES)),
                               trace=_trace, **trace_kwargs)
    out = np.stack(
        [res.results[i]["out"].reshape(S_DIM, C_DIM, H, W) for i in range(B)],
        axis=0,
    ).astype(np.float32)
    if _trace:
        return out, res
    return out


# revision 5
# speedup vs baseline: 1.0922x; 1.0922x over previous
"""Trainium2 Bass kernel for nn_HeatmapEncoder.

Math per (b, s, c) and per coordinate set (gaze, hand):
    g = exp(-((gx-cx)^2 + (gy-cy)^2) / (2 sigma^2))   on a 336x336 grid
    g = g / (sum(g) + eps)            (zeroed when cx+cy <= 0)
    unified = g_gaze + g_hand
    out = unified / (max(unified) + eps)

The Gaussian is separable, so each unified map is rank-2: one K=2 bf16
matmul per 512-wide PSUM chunk (rows: (y_gaze, x_gaze), (y_hand,
x_hand); plain bf16 rounding, rel err ~4e-3 vs the 2e-2 gate).
Sum-normalization is folded into the y factors.

The peak is computed ANALYTICALLY before any map exists: for two
equal-sigma isotropic Gaussians the continuous max lies on the segment
between the two centers, so sampling
    f(t) = A exp(-D^2 t^2 / 2s^2) + B exp(-D^2 (1-t)^2 / 2s^2)
at SEG points (A, B = per-set amplitudes, D^2 = |c1-c2|^2 from the
host) bounds the grid peak to ~0.3%.  Gaze rows sample t^2, hand rows
(1-t)^2; a [64,32] pairing matmul adds the two sets per map, and a
[32,32] identity matmul broadcasts peak+eps across partitions — no
cross-partition DMA.  Each map is then drained ONCE from PSUM with a
fused scale (1/(peak+eps)) split across DVE (c=0) and ACT (c=1,2) so
the PSUM slot frees fast, then DMA'd out on the sync queue (the only
deep DMA ring; gpsimd's ring forces ~1us DRAIN stalls when overused).

Factors are computed on the natural [64, W] rows and scattered into
the 32-aligned PE layout with direct SBUF->SBUF DMAs (no DRAM bounce):
map j = 4*b + q keeps its 2 factor rows at partitions 32q, 32q+1, free
block b.  Map rows are interleaved y = 3*p + c so each map is a single
contiguous DRAM range for the output DMA.

Sharding: pure data parallel over batch B=8 across the 8 cores.
"""

import functools
from contextlib import ExitStack

import numpy as np
import ml_dtypes

try:
    import concourse.bass as bass
except ImportError:  # pragma: no cover
    import sys

    sys.path.insert(0, "/opt/trn_rl_repo")
    import concourse.bass as bass

import concourse.tile as tile
from concourse import bacc, mybir
from concourse.bass_utils import run_bass_kernel_spmd

H = W = 336
P = 112  # partitions per y-chunk; y = 3*p + c  (c in 0..2)
NCH = 3
S_DIM, C_DIM = 8, 4
NMAPS = S_DIM * C_DIM  # 32 maps per core
NR = 2 * NMAPS  # 64 factor rows (map-major, gaze/hand interleaved)
NB = 8  # free blocks in the aligned factor layout (map j = 4*b + q)
N_CORES = 8
SIGMA = 10.0 / 336.0
EXP_SCALE = -1.0 / (2.0 * SIGMA * SIGMA)
EPS = 1e-6
SEG = 512  # segment samples for the analytic peak

F32 = mybir.dt.float32
BF16 = mybir.dt.bfloat16
AF = mybir.ActivationFunctionType
ALU = mybir.AluOpType
AX = mybir.AxisListType


def _emit(nc, tc, ctx, negc_in, dsq_in, out_t, grid_const, ts_const,
          pair_const, eye_const):
    const = ctx.enter_context(tc.tile_pool(name="const", bufs=1))
    fact = ctx.enter_context(tc.tile_pool(name="fact", bufs=1))
    ffac = ctx.enter_context(tc.tile_pool(name="ffac", bufs=1))
    small = ctx.enter_context(tc.tile_pool(name="small", bufs=2))
    sstage = ctx.enter_context(tc.tile_pool(name="sstage", bufs=16))
    pmap = ctx.enter_context(tc.tile_pool(name="pmap", bufs=2, space="PSUM"))
    ppk = ctx.enter_context(tc.tile_pool(name="ppk", bufs=1, space="PSUM"))

    # ---- early ACT table preload via dummy exp on a memset tile ----
    dum = small.tile([1, 16], F32, tag="dum")
    nc.vector.memset(dum[:], 0.0)
    dum2 = small.tile([1, 16], F32, tag="dum2")
    nc.scalar.activation(dum2[:], dum[:], AF.Exp, bias=0.0, scale=1.0)
    ONES = const.tile([NMAPS, P], BF16)
    nc.gpsimd.memset(ONES[:], 1.0)

    # ---- constants / inputs ----
    # sync: G, NC2 gate the factor chain; peak-chain consts on gpsimd
    G = const.tile([NR, W], F32)
    nc.sync.dma_start(G[:], grid_const.ap())
    NC2 = const.tile([NR, 2], F32)
    nc.sync.dma_start(NC2[:], negc_in.ap())
    PAIR = const.tile([NR, NMAPS], BF16)
    nc.gpsimd.dma_start(PAIR[:], pair_const.ap())
    EYE = const.tile([NMAPS, NMAPS], BF16)
    nc.gpsimd.dma_start(EYE[:], eye_const.ap())
    TS = const.tile([NR, SEG], F32)
    nc.gpsimd.dma_start(TS[:], ts_const.ap())
    DSQ = const.tile([NR, 1], F32)
    nc.gpsimd.dma_start(DSQ[:], dsq_in.ap())

    # ---- 1-D gaussian factors, [64, 336] fp32; row sums via ACT accum ----
    sx = small.tile([NR, 1], F32, tag="sx")
    sy = small.tile([NR, 1], F32, tag="sy")
    sqx = fact.tile([NR, W], F32)
    nc.scalar.activation(sqx[:], G[:], AF.Square, bias=NC2[:, 0:1], scale=1.0)
    fxv = fact.tile([NR, W], F32)
    nc.scalar.activation(fxv[:], sqx[:], AF.Exp, bias=0.0, scale=EXP_SCALE,
                         accum_out=sx[:])
    sqy = fact.tile([NR, W], F32)
    nc.scalar.activation(sqy[:], G[:], AF.Square, bias=NC2[:, 1:2], scale=1.0)
    fyv = fact.tile([NR, W], F32)
    nc.scalar.activation(fyv[:], sqy[:], AF.Exp, bias=0.0, scale=EXP_SCALE,
                         accum_out=sy[:])

    # x side: bf16 round only (K=2), scatter straight to the PE layout
    xh = fact.tile([NR, W], BF16)
    nc.vector.tensor_copy(xh[:], fxv[:])

    # normalization scale a = valid / (Sx*Sy + eps) folded into y factors
    ss = small.tile([NR, 1], F32, tag="ss")
    nc.vector.tensor_mul(ss[:], sx[:], sy[:])
    sse = small.tile([NR, 1], F32, tag="sse")
    nc.vector.tensor_scalar_add(sse[:], ss[:], EPS)
    rec = small.tile([NR, 1], F32, tag="rec")
    nc.vector.reciprocal(rec[:], sse[:])
    vs = small.tile([NR, 1], F32, tag="vs")
    nc.vector.tensor_add(vs[:], NC2[:, 0:1], NC2[:, 1:2])
    vm = small.tile([NR, 1], F32, tag="vm")  # valid: (-cx)+(-cy) < 0
    nc.vector.tensor_scalar(vm[:], vs[:], 0.0, None, op0=ALU.is_lt)
    av = small.tile([NR, 1], F32, tag="av")
    nc.vector.tensor_mul(av[:], rec[:], vm[:])
    fys = fact.tile([NR, W], F32)
    nc.scalar.mul(fys[:], fyv[:], av[:, 0:1])
    yh = fact.tile([NR, W], BF16)
    nc.vector.tensor_copy(yh[:], fys[:])

    # ---- analytic peak chain (no map read-back, no DRAM hop) ----
    # f_row(t_i) = av_row * exp(EXP_SCALE * D^2 * {t^2 | (1-t)^2})
    us = fact.tile([NR, SEG], F32)
    nc.vector.tensor_scalar_mul(us[:], TS[:], DSQ[:, 0:1])
    ee = fact.tile([NR, SEG], F32)
    nc.scalar.activation(ee[:], us[:], AF.Exp, bias=0.0, scale=EXP_SCALE)
    pp = fact.tile([NR, SEG], BF16)
    nc.vector.tensor_scalar_mul(pp[:], ee[:], av[:, 0:1])
    # pair matmul: psum[j, i] = f_gaze(2j) + f_hand(2j+1) = unified on segment
    pk_ps = ppk.tile([NMAPS, SEG], F32, tag="pkps")
    nc.tensor.matmul(pk_ps[:], PAIR[:], pp[:], start=True, stop=True,
                     tile_position=(0, 0))
    pk = small.tile([NMAPS, 1], F32, tag="pk")
    nc.vector.reduce_max(pk[:], pk_ps[:], axis=AX.X)
    pke = small.tile([NMAPS, 1], F32, tag="pke")
    nc.vector.tensor_scalar_add(pke[:], pk[:], EPS)
    # broadcast pke across partitions via identity matmul, then 1/x
    pkb = small.tile([NMAPS, P], BF16, tag="pkb")
    nc.vector.tensor_scalar_mul(pkb[:], ONES[:], pke[:, 0:1])
    rg_ps = ppk.tile([P, NMAPS], F32, tag="rgps")
    nc.tensor.matmul(rg_ps[:], pkb[:], EYE[:], start=True, stop=True,
                     tile_position=(0, 0))
    RG = const.tile([P, NMAPS], F32)
    nc.vector.reciprocal(RG[:], rg_ps[:])

    # ---- scatter factors into the 32-aligned 2-row layout, SBUF->SBUF ----
    # dest rows 32q+t  <-  factor row 8b+2q+t, free block b
    FYq = [ffac.tile([128, NB, W], BF16, name=f"FY{q}", tag=f"fy{q}")
           for q in range(4)]
    FXq = [ffac.tile([128, NB, W], BF16, name=f"FX{q}", tag=f"fx{q}")
           for q in range(4)]
    for q in range(4):
        for t in range(2):
            nc.sync.dma_start(FYq[q][32 * q + t:32 * q + t + 1, :, :],
                              yh[2 * q + t::8, :])
            nc.gpsimd.dma_start(FXq[q][32 * q + t:32 * q + t + 1, :, :],
                                xh[2 * q + t::8, :])

    # DRAM view matching stage layout: out[m, y, x], y = 3p+c, z = 336c+x
    dview = out_t.ap().rearrange("m (p c) x -> p m (c x)", p=P)

    def map_matmuls(j, pt):
        q, b = j % 4, j // 4
        rhs = FXq[q][32 * q:32 * q + 2, b, :]
        for cix in range(NCH):
            lhsT = FYq[q][32 * q:32 * q + 2, b, cix::3]
            nc.tensor.matmul(pt[:, cix * 512:cix * 512 + W], lhsT, rhs,
                             start=True, stop=True, tile_position=(32 * q, 0))

    for j in range(NMAPS):
        pt = pmap.tile([P, NCH * 512], F32, tag="pmap")
        map_matmuls(j, pt)
        pview = pt[:].rearrange("p (c z) -> p c z", c=NCH)[:, :, 0:W]
        st = sstage.tile([P, NCH * W], F32, tag="sst")
        sview = st[:].rearrange("p (c x) -> p c x", c=NCH)
        # fused scale-drain split across both engines frees the slot fast
        nc.vector.tensor_scalar_mul(sview[:, 0:1, :], pview[:, 0:1, :],
                                    RG[:, j:j + 1])
        nc.scalar.mul(sview[:, 1:3, :], pview[:, 1:3, :], RG[:, j:j + 1])
        nc.sync.dma_start(dview[:, j:j + 1, :], st[:])


@functools.lru_cache(maxsize=1)
def _build():
    nc = bacc.Bacc("TRN2", target_bir_lowering=False, debug=False)
    negc_in = nc.dram_tensor("negc", [NR, 2], F32, kind="ExternalInput")
    dsq_in = nc.dram_tensor("dsq", [NR, 1], F32, kind="ExternalInput")
    out_t = nc.dram_tensor("out", [NMAPS, H, W], F32, kind="ExternalOutput")

    grid = (np.arange(W, dtype=np.float64) / (W - 1)).astype(np.float32)
    grid_const = nc.inline_tensor(np.tile(grid, (NR, 1)), name="gridc")

    t = np.arange(SEG, dtype=np.float64) / (SEG - 1)
    ts = np.empty((NR, SEG), dtype=np.float32)
    ts[0::2] = (t ** 2).astype(np.float32)
    ts[1::2] = ((1.0 - t) ** 2).astype(np.float32)
    ts_const = nc.inline_tensor(ts, name="tsc")

    pair = np.zeros((NR, NMAPS), dtype=ml_dtypes.bfloat16)
    pair[np.arange(NR), np.arange(NR) // 2] = 1
    pair_const = nc.inline_tensor(pair, name="pairc")
    eye_const = nc.inline_tensor(np.eye(NMAPS, dtype=ml_dtypes.bfloat16),
                                 name="eyec")

    with tile.TileContext(nc) as tc, ExitStack() as ctx:
        _emit(nc, tc, ctx, negc_in, dsq_in, out_t, grid_const, ts_const,
              pair_const, eye_const)
    nc.compile()
    return nc


def _in_map_for(gaze, hand, b):
    cg = np.asarray(gaze[b], dtype=np.float32).reshape(NMAPS, 2)
    ch = np.asarray(hand[b], dtype=np.float32).reshape(NMAPS, 2)
    inter = np.stack([cg, ch], axis=1).reshape(NR, 2)  # row 2*j + t
    d2 = ((cg - ch) ** 2).sum(-1)  # |c_gaze - c_hand|^2 per map
    dsq = np.repeat(d2, 2)[:, None].astype(np.float32)
    return {"negc": np.ascontiguousarray(-inter),
            "dsq": np.ascontiguousarray(dsq)}


def kernel(gaze_coords, hand_coords, _trace=False, **trace_kwargs):
    gaze_coords = np.asarray(gaze_coords, dtype=np.float32)
    hand_coords = np.asarray(hand_coords, dtype=np.float32)
    B = gaze_coords.shape[0]
    assert B == N_CORES, f"expected batch {N_CORES}, got {B}"
    nc = _build()
    in_maps = [_in_map_for(gaze_coords, hand_coords, b) for b in range(B)]
    res = run_bass_kernel_spmd(nc, in_maps, list(range(N_CORES)),
                               trace=_trace, **trace_kwargs)
    out = np.stack(
        [res.results[i]["out"].reshape(S_DIM, C_DIM, H, W) for i in range(B)],
        axis=0,
    ).astype(np.float32)
    if _trace:
        return out, res
    return out


# revision 9
# speedup vs baseline: 1.1134x; 1.0194x over previous
"""Trainium2 Bass kernel for nn_HeatmapEncoder.

Math per (b, s, c) and per coordinate set (gaze, hand):
    g = exp(-((gx-cx)^2 + (gy-cy)^2) / (2 sigma^2))   on a 336x336 grid
    g = g / (sum(g) + eps)            (zeroed when cx+cy <= 0)
    unified = g_gaze + g_hand
    out = unified / (max(unified) + eps)

The Gaussian is separable, so each unified map is rank-2: one K=2 bf16
matmul per 512-wide PSUM chunk (rows: (y_gaze, x_gaze), (y_hand,
x_hand); plain bf16 rounding, rel err ~4e-3 vs the 2e-2 gate).
Sum-normalization is folded into the y factors.

The peak is computed ANALYTICALLY before any map exists: for two
equal-sigma isotropic Gaussians the continuous max lies on the segment
between the two centers, so sampling
    f(t) = A exp(-D^2 t^2 / 2s^2) + B exp(-D^2 (1-t)^2 / 2s^2)
at SEG points (A, B = per-set amplitudes, D^2 = |c1-c2|^2 from the
host) bounds the grid peak to ~0.3%.  Gaze rows sample t^2, hand rows
(1-t)^2; a [64,32] pairing matmul adds the two sets per map, and a
[32,32] identity matmul broadcasts peak+eps across partitions — no
cross-partition DMA.  Each map is then drained ONCE from PSUM with a
fused scale (1/(peak+eps)) split across DVE (c=0) and ACT (c=1,2) so
the PSUM slot frees fast, then DMA'd out on the sync queue (the only
deep DMA ring; gpsimd's ring forces ~1us DRAIN stalls when overused).

Factors are computed on the natural [64, W] rows and scattered into
the 32-aligned PE layout with direct SBUF->SBUF DMAs (no DRAM bounce):
map j = 4*b + q keeps its 2 factor rows at partitions 32q, 32q+1, free
block b.  Map rows are interleaved y = 3*p + c so each map is a single
contiguous DRAM range for the output DMA.

Sharding: pure data parallel over batch B=8 across the 8 cores.
"""

import functools
from contextlib import ExitStack

import numpy as np
import ml_dtypes

try:
    import concourse.bass as bass
except ImportError:  # pragma: no cover
    import sys

    sys.path.insert(0, "/opt/trn_rl_repo")
    import concourse.bass as bass

import concourse.tile as tile
from concourse import bacc, mybir
from concourse.bass_utils import run_bass_kernel_spmd

H = W = 336
P = 112  # partitions per y-chunk; y = 3*p + c  (c in 0..2)
NCH = 3
S_DIM, C_DIM = 8, 4
NMAPS = S_DIM * C_DIM  # 32 maps per core
NR = 2 * NMAPS  # 64 factor rows (map-major, gaze/hand interleaved)
NB = 8  # free blocks in the aligned factor layout (map j = 4*b + q)
N_CORES = 8
SIGMA = 10.0 / 336.0
EXP_SCALE = -1.0 / (2.0 * SIGMA * SIGMA)
EPS = 1e-6
SEG = 512  # segment samples for the analytic peak

F32 = mybir.dt.float32
BF16 = mybir.dt.bfloat16
AF = mybir.ActivationFunctionType
ALU = mybir.AluOpType
AX = mybir.AxisListType


def _emit(nc, tc, ctx, negc_in, dsq_in, out_t, grid_const, ts_const,
          pair_const, eye_const):
    const = ctx.enter_context(tc.tile_pool(name="const", bufs=1))
    fact = ctx.enter_context(tc.tile_pool(name="fact", bufs=1))
    ffac = ctx.enter_context(tc.tile_pool(name="ffac", bufs=1))
    small = ctx.enter_context(tc.tile_pool(name="small", bufs=2))
    sst_pools = {
        1: ctx.enter_context(tc.tile_pool(name="sst1", bufs=8)),
        2: ctx.enter_context(tc.tile_pool(name="sst2", bufs=4)),
        4: ctx.enter_context(tc.tile_pool(name="sst4", bufs=3)),
    }
    pmap = ctx.enter_context(tc.tile_pool(name="pmap", bufs=2, space="PSUM"))
    ppk = ctx.enter_context(tc.tile_pool(name="ppk", bufs=1, space="PSUM"))

    # ---- early ACT table preload via dummy exp on a memset tile ----
    dum = small.tile([1, 16], F32, tag="dum")
    nc.vector.memset(dum[:], 0.0)
    dum2 = small.tile([1, 16], F32, tag="dum2")
    nc.scalar.activation(dum2[:], dum[:], AF.Exp, bias=0.0, scale=1.0)
    ONES = const.tile([NMAPS, P], BF16)
    nc.gpsimd.memset(ONES[:], 1.0)

    # ---- constants / inputs ----
    # sync: NC2 (tiny, first) + G gate the factor chain; rest on gpsimd
    NC2 = const.tile([NR, 2], F32)
    nc.sync.dma_start(NC2[:], negc_in.ap())
    G = const.tile([NR, W], F32)
    nc.sync.dma_start(G[:], grid_const.ap())
    PAIR = const.tile([NR, NMAPS], BF16)
    nc.gpsimd.dma_start(PAIR[:], pair_const.ap())
    EYE = const.tile([NMAPS, NMAPS], BF16)
    nc.gpsimd.dma_start(EYE[:], eye_const.ap())
    TS = const.tile([NR, SEG], F32)
    nc.gpsimd.dma_start(TS[:], ts_const.ap())
    DSQ = const.tile([NR, 1], F32)
    nc.gpsimd.dma_start(DSQ[:], dsq_in.ap())

    # ---- 1-D gaussian factors, [64, 336] fp32; row sums via ACT accum ----
    # y side first: it gates the FY scatters -> first matmul.
    # Normalization split per side: y carries valid/Sy, x carries 1/Sx
    # (the reference's +eps on Sx*Sy is ~1e-8 relative - dropped).
    sx = small.tile([NR, 1], F32, tag="sx")
    sy = small.tile([NR, 1], F32, tag="sy")
    sqy = fact.tile([NR, W], F32)
    nc.scalar.activation(sqy[:], G[:], AF.Square, bias=NC2[:, 1:2], scale=1.0)
    fyv = fact.tile([NR, W], F32)
    nc.scalar.activation(fyv[:], sqy[:], AF.Exp, bias=0.0, scale=EXP_SCALE,
                         accum_out=sy[:])
    sqx = fact.tile([NR, W], F32)
    nc.scalar.activation(sqx[:], G[:], AF.Square, bias=NC2[:, 0:1], scale=1.0)
    fxv = fact.tile([NR, W], F32)
    nc.scalar.activation(fxv[:], sqx[:], AF.Exp, bias=0.0, scale=EXP_SCALE,
                         accum_out=sx[:])

    # off the critical path: peak-segment arg + validity mask
    us = fact.tile([NR, SEG], F32)
    nc.vector.tensor_scalar_mul(us[:], TS[:], DSQ[:, 0:1])
    vs = small.tile([NR, 1], F32, tag="vs")
    nc.vector.tensor_add(vs[:], NC2[:, 0:1], NC2[:, 1:2])
    vm = small.tile([NR, 1], F32, tag="vm")  # valid: (-cx)+(-cy) < 0
    nc.vector.tensor_scalar(vm[:], vs[:], 0.0, None, op0=ALU.is_lt)

    # y factors: bf16 with valid/Sy folded in (critical path)
    ry = small.tile([NR, 1], F32, tag="ry")
    nc.vector.reciprocal(ry[:], sy[:])
    rv = small.tile([NR, 1], F32, tag="rv")
    nc.vector.tensor_mul(rv[:], ry[:], vm[:])
    yh = fact.tile([NR, W], BF16)
    nc.vector.tensor_scalar_mul(yh[:], fyv[:], rv[:, 0:1])

    # x factors: bf16 with 1/Sx folded in
    rx = small.tile([NR, 1], F32, tag="rx")
    nc.vector.reciprocal(rx[:], sx[:])
    xh = fact.tile([NR, W], BF16)
    nc.vector.tensor_scalar_mul(xh[:], fxv[:], rx[:, 0:1])

    # ---- analytic peak chain (no map read-back, no DRAM hop) ----
    # f_row(t_i) = av_row * exp(EXP_SCALE * D^2 * {t^2 | (1-t)^2})
    am = small.tile([NR, 1], F32, tag="am")
    nc.vector.tensor_mul(am[:], rx[:], ry[:])
    av = small.tile([NR, 1], F32, tag="av")
    nc.vector.tensor_mul(av[:], am[:], vm[:])
    ee = fact.tile([NR, SEG], F32)
    nc.scalar.activation(ee[:], us[:], AF.Exp, bias=0.0, scale=EXP_SCALE)
    pp = fact.tile([NR, SEG], BF16)
    nc.vector.tensor_scalar_mul(pp[:], ee[:], av[:, 0:1])
    # pair matmul: psum[j, i] = f_gaze(2j) + f_hand(2j+1) = unified on segment
    pk_ps = ppk.tile([NMAPS, SEG], F32, tag="pkps")
    nc.tensor.matmul(pk_ps[:], PAIR[:], pp[:], start=True, stop=True,
                     tile_position=(0, 0))
    pk = small.tile([NMAPS, 1], F32, tag="pk")
    nc.vector.reduce_max(pk[:], pk_ps[:], axis=AX.X)
    pke = small.tile([NMAPS, 1], F32, tag="pke")
    nc.vector.tensor_scalar_add(pke[:], pk[:], EPS)
    # broadcast pke across partitions via identity matmul, then 1/x
    pkb = small.tile([NMAPS, P], BF16, tag="pkb")
    nc.vector.tensor_scalar_mul(pkb[:], ONES[:], pke[:, 0:1])
    rg_ps = ppk.tile([P, NMAPS], F32, tag="rgps")
    nc.tensor.matmul(rg_ps[:], pkb[:], EYE[:], start=True, stop=True,
                     tile_position=(0, 0))
    RG = const.tile([P, NMAPS], F32)
    nc.vector.reciprocal(RG[:], rg_ps[:])

    # ---- scatter factors into the 32-aligned 2-row layout, SBUF->SBUF ----
    # dest rows 32q+t  <-  factor row 8b+2q+t, free block b
    FYq = [ffac.tile([128, NB, W], BF16, name=f"FY{q}", tag=f"fy{q}")
           for q in range(4)]
    FXq = [ffac.tile([128, NB, W], BF16, name=f"FX{q}", tag=f"fx{q}")
           for q in range(4)]
    for q in range(4):
        for t in range(2):
            nc.sync.dma_start(FYq[q][32 * q + t:32 * q + t + 1, :, :],
                              yh[2 * q + t::8, :])
            nc.gpsimd.dma_start(FXq[q][32 * q + t:32 * q + t + 1, :, :],
                                xh[2 * q + t::8, :])

    # DRAM view matching stage layout: out[m, y, x], y = 3p+c, z = 336c+x
    dview = out_t.ap().rearrange("m (p c) x -> p m (c x)", p=P)

    def map_matmuls(j, pt):
        q, b = j % 4, j // 4
        rhs = FXq[q][32 * q:32 * q + 2, b, :]
        for cix in range(NCH):
            lhsT = FYq[q][32 * q:32 * q + 2, b, cix::3]
            nc.tensor.matmul(pt[:, cix * 512:cix * 512 + W], lhsT, rhs,
                             start=True, stop=True, tile_position=(32 * q, 0))

    # output DMA groups: singles while the pipe fills, then pairs/quads
    # (fewer, larger DMAs keep all 16 DMA engines fed between maps)
    j = 0
    for g in (1, 1, 1, 1, 1, 1, 1, 1, 2, 2, 2, 2, 4, 4, 4, 4):
        st = sst_pools[g].tile([P, g, NCH * W], F32, tag=f"sst{g}")
        for k in range(g):
            pt = pmap.tile([P, NCH * 512], F32, tag="pmap")
            map_matmuls(j + k, pt)
            pview = pt[:].rearrange("p (c z) -> p c z", c=NCH)[:, :, 0:W]
            sview = st[:, k, :].rearrange("p (c x) -> p c x", c=NCH)
            # fused scale-drain split across both engines frees the slot
            nc.vector.tensor_scalar_mul(sview[:, 0:1, :], pview[:, 0:1, :],
                                        RG[:, j + k:j + k + 1])
            nc.scalar.mul(sview[:, 1:3, :], pview[:, 1:3, :],
                          RG[:, j + k:j + k + 1])
        nc.sync.dma_start(dview[:, j:j + g, :], st[:])
        j += g


@functools.lru_cache(maxsize=1)
def _build():
    nc = bacc.Bacc("TRN2", target_bir_lowering=False, debug=False)
    negc_in = nc.dram_tensor("negc", [NR, 2], F32, kind="ExternalInput")
    dsq_in = nc.dram_tensor("dsq", [NR, 1], F32, kind="ExternalInput")
    out_t = nc.dram_tensor("out", [NMAPS, H, W], F32, kind="ExternalOutput")

    grid = (np.arange(W, dtype=np.float64) / (W - 1)).astype(np.float32)
    grid_const = nc.inline_tensor(np.tile(grid, (NR, 1)), name="gridc")

    t = np.arange(SEG, dtype=np.float64) / (SEG - 1)
    ts = np.empty((NR, SEG), dtype=np.float32)
    ts[0::2] = (t ** 2).astype(np.float32)
    ts[1::2] = ((1.0 - t) ** 2).astype(np.float32)
    ts_const = nc.inline_tensor(ts, name="tsc")

    pair = np.zeros((NR, NMAPS), dtype=ml_dtypes.bfloat16)
    pair[np.arange(NR), np.arange(NR) // 2] = 1
    pair_const = nc.inline_tensor(pair, name="pairc")
    eye_const = nc.inline_tensor(np.eye(NMAPS, dtype=ml_dtypes.bfloat16),
                                 name="eyec")

    with tile.TileContext(nc) as tc, ExitStack() as ctx:
        _emit(nc, tc, ctx, negc_in, dsq_in, out_t, grid_const, ts_const,
              pair_const, eye_const)
    nc.compile()
    return nc


def _in_map_for(gaze, hand, b):
    cg = np.asarray(gaze[b], dtype=np.float32).reshape(NMAPS, 2)
    ch = np.asarray(hand[b], dtype=np.float32).reshape(NMAPS, 2)
    inter = np.stack([cg, ch], axis=1).reshape(NR, 2)  # row 2*j + t
    d2 = ((cg - ch) ** 2).sum(-1)  # |c_gaze - c_hand|^2 per map
    dsq = np.repeat(d2, 2)[:, None].astype(np.float32)
    return {"negc": np.ascontiguousarray(-inter),
            "dsq": np.ascontiguousarray(dsq)}


def kernel(gaze_coords, hand_coords, _trace=False, **trace_kwargs):
    gaze_coords = np.asarray(gaze_coords, dtype=np.float32)
    hand_coords = np.asarray(hand_coords, dtype=np.float32)
    B = gaze_coords.shape[0]
    assert B == N_CORES, f"expected batch {N_CORES}, got {B}"
    nc = _build()
    in_maps = [_in_map_for(gaze_coords, hand_coords, b) for b in range(B)]
    res = run_bass_kernel_spmd(nc, in_maps, list(range(N_CORES)),
                               trace=_trace, **trace_kwargs)
    out = np.stack(
        [res.results[i]["out"].reshape(S_DIM, C_DIM, H, W) for i in range(B)],
        axis=0,
    ).astype(np.float32)
    if _trace:
        return out, res
    return out


# revision 13
# speedup vs baseline: 1.1187x; 1.0047x over previous
"""Trainium2 Bass kernel for nn_HeatmapEncoder.

Math per (b, s, c) and per coordinate set (gaze, hand):
    g = exp(-((gx-cx)^2 + (gy-cy)^2) / (2 sigma^2))   on a 336x336 grid
    g = g / (sum(g) + eps)            (zeroed when cx+cy <= 0)
    unified = g_gaze + g_hand
    out = unified / (max(unified) + eps)

The Gaussian is separable, so each unified map is rank-2: one K=2 bf16
matmul per 512-wide PSUM chunk (rows: (y_gaze, x_gaze), (y_hand,
x_hand); plain bf16 rounding, rel err ~4e-3 vs the 2e-2 gate).
Sum-normalization is folded into the y factors.

The peak is computed ANALYTICALLY before any map exists: for two
equal-sigma isotropic Gaussians the continuous max lies on the segment
between the two centers, so sampling
    f(t) = A exp(-D^2 t^2 / 2s^2) + B exp(-D^2 (1-t)^2 / 2s^2)
at SEG points (A, B = per-set amplitudes, D^2 = |c1-c2|^2 from the
host) bounds the grid peak to ~0.3%.  Gaze rows sample t^2, hand rows
(1-t)^2; a [64,32] pairing matmul adds the two sets per map, and a
[32,32] identity matmul broadcasts peak+eps across partitions — no
cross-partition DMA.  Each map is then drained ONCE from PSUM with a
fused scale (1/(peak+eps)) split across DVE (c=0) and ACT (c=1,2) so
the PSUM slot frees fast, then DMA'd out on the sync queue (the only
deep DMA ring; gpsimd's ring forces ~1us DRAIN stalls when overused).

Factors are computed on the natural [64, W] rows and scattered into
the 32-aligned PE layout with direct SBUF->SBUF DMAs (no DRAM bounce):
map j = 4*b + q keeps its 2 factor rows at partitions 32q, 32q+1, free
block b.  Map rows are interleaved y = 3*p + c so each map is a single
contiguous DRAM range for the output DMA.

Sharding: pure data parallel over batch B=8 across the 8 cores.
"""

import functools
from contextlib import ExitStack

import numpy as np
import ml_dtypes

try:
    import concourse.bass as bass
except ImportError:  # pragma: no cover
    import sys

    sys.path.insert(0, "/opt/trn_rl_repo")
    import concourse.bass as bass

import concourse.tile as tile
from concourse import bacc, mybir
from concourse.bass_utils import run_bass_kernel_spmd

H = W = 336
P = 112  # partitions per y-chunk; y = 3*p + c  (c in 0..2)
NCH = 3
S_DIM, C_DIM = 8, 4
NMAPS = S_DIM * C_DIM  # 32 maps per core
NR = 2 * NMAPS  # 64 factor rows (map-major, gaze/hand interleaved)
NB = 8  # free blocks in the aligned factor layout (map j = 4*b + q)
N_CORES = 8
SIGMA = 10.0 / 336.0
EXP_SCALE = -1.0 / (2.0 * SIGMA * SIGMA)
EPS = 1e-6
SEG = 512  # segment samples for the analytic peak

F32 = mybir.dt.float32
BF16 = mybir.dt.bfloat16
AF = mybir.ActivationFunctionType
ALU = mybir.AluOpType
AX = mybir.AxisListType


def _emit(nc, tc, ctx, negc_in, dsq_in, out_t, grid_const, ts_const,
          pair_const, eye_const):
    const = ctx.enter_context(tc.tile_pool(name="const", bufs=1))
    fact = ctx.enter_context(tc.tile_pool(name="fact", bufs=1))
    ffac = ctx.enter_context(tc.tile_pool(name="ffac", bufs=1))
    small = ctx.enter_context(tc.tile_pool(name="small", bufs=2))
    sst_pools = {
        1: ctx.enter_context(tc.tile_pool(name="sst1", bufs=8)),
        2: ctx.enter_context(tc.tile_pool(name="sst2", bufs=4)),
        4: ctx.enter_context(tc.tile_pool(name="sst4", bufs=4)),
    }
    pmap = ctx.enter_context(tc.tile_pool(name="pmap", bufs=2, space="PSUM"))
    ppk = ctx.enter_context(tc.tile_pool(name="ppk", bufs=1, space="PSUM"))

    # ---- early ACT table preload via dummy exp on a memset tile ----
    dum = small.tile([1, 16], F32, tag="dum")
    nc.vector.memset(dum[:], 0.0)
    dum2 = small.tile([1, 16], F32, tag="dum2")
    nc.scalar.activation(dum2[:], dum[:], AF.Exp, bias=0.0, scale=1.0)
    ONES = const.tile([NMAPS, P], BF16)
    nc.gpsimd.memset(ONES[:], 1.0)

    # ---- constants / inputs ----
    # sync: NC2 (tiny, first) + G gate the factor chain; rest on gpsimd
    NC2 = const.tile([NR, 2], F32)
    nc.sync.dma_start(NC2[:], negc_in.ap())
    G = const.tile([NR, W], F32)
    nc.sync.dma_start(G[:], grid_const.ap())
    PAIR = const.tile([NR, NMAPS], BF16)
    nc.gpsimd.dma_start(PAIR[:], pair_const.ap())
    EYE = const.tile([NMAPS, NMAPS], BF16)
    nc.gpsimd.dma_start(EYE[:], eye_const.ap())
    TS = const.tile([NR, SEG], F32)
    nc.gpsimd.dma_start(TS[:], ts_const.ap())
    DSQ = const.tile([NR, 1], F32)
    nc.gpsimd.dma_start(DSQ[:], dsq_in.ap())

    # ---- 1-D gaussian factors, [64, 336] fp32; row sums via ACT accum ----
    # y side first: it gates the FY scatters -> first matmul.
    # Normalization split per side: y carries valid/Sy, x carries 1/Sx
    # (the reference's +eps on Sx*Sy is ~1e-8 relative - dropped).
    sx = small.tile([NR, 1], F32, tag="sx")
    sy = small.tile([NR, 1], F32, tag="sy")
    sqy = fact.tile([NR, W], F32)
    nc.scalar.activation(sqy[:], G[:], AF.Square, bias=NC2[:, 1:2], scale=1.0)
    fyv = fact.tile([NR, W], F32)
    nc.scalar.activation(fyv[:], sqy[:], AF.Exp, bias=0.0, scale=EXP_SCALE,
                         accum_out=sy[:])
    sqx = fact.tile([NR, W], F32)
    nc.scalar.activation(sqx[:], G[:], AF.Square, bias=NC2[:, 0:1], scale=1.0)
    fxv = fact.tile([NR, W], F32)
    nc.scalar.activation(fxv[:], sqx[:], AF.Exp, bias=0.0, scale=EXP_SCALE,
                         accum_out=sx[:])

    # off the critical path: peak-segment arg + validity mask
    us = fact.tile([NR, SEG], F32)
    nc.vector.tensor_scalar_mul(us[:], TS[:], DSQ[:, 0:1])
    vs = small.tile([NR, 1], F32, tag="vs")
    nc.vector.tensor_add(vs[:], NC2[:, 0:1], NC2[:, 1:2])
    vm = small.tile([NR, 1], F32, tag="vm")  # valid: (-cx)+(-cy) < 0
    nc.vector.tensor_scalar(vm[:], vs[:], 0.0, None, op0=ALU.is_lt)

    # y factors: bf16 with valid/Sy folded in (critical path)
    ry = small.tile([NR, 1], F32, tag="ry")
    nc.vector.reciprocal(ry[:], sy[:])
    rv = small.tile([NR, 1], F32, tag="rv")
    nc.vector.tensor_mul(rv[:], ry[:], vm[:])
    yh = fact.tile([NR, W], BF16)
    nc.vector.tensor_scalar_mul(yh[:], fyv[:], rv[:, 0:1])

    # x factors: bf16 with 1/Sx folded in
    rx = small.tile([NR, 1], F32, tag="rx")
    nc.vector.reciprocal(rx[:], sx[:])
    xh = fact.tile([NR, W], BF16)
    nc.vector.tensor_scalar_mul(xh[:], fxv[:], rx[:, 0:1])

    # ---- analytic peak chain (no map read-back, no DRAM hop) ----
    # f_row(t_i) = av_row * exp(EXP_SCALE * D^2 * {t^2 | (1-t)^2})
    am = small.tile([NR, 1], F32, tag="am")
    nc.vector.tensor_mul(am[:], rx[:], ry[:])
    av = small.tile([NR, 1], F32, tag="av")
    nc.vector.tensor_mul(av[:], am[:], vm[:])
    ee = fact.tile([NR, SEG], F32)
    nc.scalar.activation(ee[:], us[:], AF.Exp, bias=0.0, scale=EXP_SCALE)
    pp = fact.tile([NR, SEG], BF16)
    nc.vector.tensor_scalar_mul(pp[:], ee[:], av[:, 0:1])
    # pair matmul: psum[j, i] = f_gaze(2j) + f_hand(2j+1) = unified on segment
    pk_ps = ppk.tile([NMAPS, SEG], F32, tag="pkps")
    nc.tensor.matmul(pk_ps[:], PAIR[:], pp[:], start=True, stop=True,
                     tile_position=(0, 0))
    pk = small.tile([NMAPS, 1], F32, tag="pk")
    nc.vector.reduce_max(pk[:], pk_ps[:], axis=AX.X)
    pke = small.tile([NMAPS, 1], F32, tag="pke")
    nc.vector.tensor_scalar_add(pke[:], pk[:], EPS)
    # broadcast pke across partitions via identity matmul, then 1/x
    pkb = small.tile([NMAPS, P], BF16, tag="pkb")
    nc.vector.tensor_scalar_mul(pkb[:], ONES[:], pke[:, 0:1])
    rg_ps = ppk.tile([P, NMAPS], F32, tag="rgps")
    nc.tensor.matmul(rg_ps[:], pkb[:], EYE[:], start=True, stop=True,
                     tile_position=(0, 0))
    RG = const.tile([P, NMAPS], F32)
    nc.vector.reciprocal(RG[:], rg_ps[:])

    # ---- scatter factors into the 32-aligned 2-row layout, SBUF->SBUF ----
    # (LDWEIGHTS requires quadrant-aligned partition starts, so the rows
    # must move to partitions 32q+t; per-q tiles keep deps fine-grained)
    FYq = [ffac.tile([128, NB, W], BF16, name=f"FY{q}", tag=f"fy{q}")
           for q in range(4)]
    FXq = [ffac.tile([128, NB, W], BF16, name=f"FX{q}", tag=f"fx{q}")
           for q in range(4)]
    for q in range(4):
        for t in range(2):
            nc.sync.dma_start(FYq[q][32 * q + t:32 * q + t + 1, :, :],
                              yh[2 * q + t::8, :])
            nc.gpsimd.dma_start(FXq[q][32 * q + t:32 * q + t + 1, :, :],
                                xh[2 * q + t::8, :])

    # DRAM view matching stage layout: out[m, y, x], y = 3p+c, z = 336c+x
    dview = out_t.ap().rearrange("m (p c) x -> p m (c x)", p=P)

    def map_matmuls(j, pt):
        q, b = j % 4, j // 4
        rhs = FXq[q][32 * q:32 * q + 2, b, :]
        for cix in range(NCH):
            lhsT = FYq[q][32 * q:32 * q + 2, b, cix::3]
            nc.tensor.matmul(pt[:, cix * 512:cix * 512 + W], lhsT, rhs,
                             start=True, stop=True, tile_position=(32 * q, 0))

    # output DMA groups: singles while the pipe fills, then pairs/quads
    # (fewer, larger DMAs keep all 16 DMA engines fed between maps)
    j = 0
    for g in (1, 1, 1, 1, 1, 1, 2, 2, 4, 4, 4, 4, 2, 2, 1, 1):
        st = sst_pools[g].tile([P, g, NCH * W], F32, tag=f"sst{g}")
        for k in range(g):
            pt = pmap.tile([P, NCH * 512], F32, tag="pmap")
            map_matmuls(j + k, pt)
            pview = pt[:].rearrange("p (c z) -> p c z", c=NCH)[:, :, 0:W]
            sview = st[:, k, :].rearrange("p (c x) -> p c x", c=NCH)
            # fused scale-drain split across both engines frees the slot
            nc.vector.tensor_scalar_mul(sview[:, 0:1, :], pview[:, 0:1, :],
                                        RG[:, j + k:j + k + 1])
            nc.scalar.mul(sview[:, 1:3, :], pview[:, 1:3, :],
                          RG[:, j + k:j + k + 1])
        nc.sync.dma_start(dview[:, j:j + g, :], st[:])
        j += g


@functools.lru_cache(maxsize=1)
def _build():
    nc = bacc.Bacc("TRN2", target_bir_lowering=False, debug=False)
    negc_in = nc.dram_tensor("negc", [NR, 2], F32, kind="ExternalInput")
    dsq_in = nc.dram_tensor("dsq", [NR, 1], F32, kind="ExternalInput")
    out_t = nc.dram_tensor("out", [NMAPS, H, W], F32, kind="ExternalOutput")

    grid = (np.arange(W, dtype=np.float64) / (W - 1)).astype(np.float32)
    grid_const = nc.inline_tensor(np.tile(grid, (NR, 1)), name="gridc")

    t = np.arange(SEG, dtype=np.float64) / (SEG - 1)
    ts = np.empty((NR, SEG), dtype=np.float32)
    ts[0::2] = (t ** 2).astype(np.float32)
    ts[1::2] = ((1.0 - t) ** 2).astype(np.float32)
    ts_const = nc.inline_tensor(ts, name="tsc")

    pair = np.zeros((NR, NMAPS), dtype=ml_dtypes.bfloat16)
    pair[np.arange(NR), np.arange(NR) // 2] = 1
    pair_const = nc.inline_tensor(pair, name="pairc")
    eye_const = nc.inline_tensor(np.eye(NMAPS, dtype=ml_dtypes.bfloat16),
                                 name="eyec")

    with tile.TileContext(nc) as tc, ExitStack() as ctx:
        _emit(nc, tc, ctx, negc_in, dsq_in, out_t, grid_const, ts_const,
              pair_const, eye_const)
    nc.compile()
    return nc


def _in_map_for(gaze, hand, b):
    cg = np.asarray(gaze[b], dtype=np.float32).reshape(NMAPS, 2)
    ch = np.asarray(hand[b], dtype=np.float32).reshape(NMAPS, 2)
    inter = np.stack([cg, ch], axis=1).reshape(NR, 2)  # row 2*j + t
    d2 = ((cg - ch) ** 2).sum(-1)  # |c_gaze - c_hand|^2 per map
    dsq = np.repeat(d2, 2)[:, None].astype(np.float32)
    return {"negc": np.ascontiguousarray(-inter),
            "dsq": np.ascontiguousarray(dsq)}


def kernel(gaze_coords, hand_coords, _trace=False, **trace_kwargs):
    gaze_coords = np.asarray(gaze_coords, dtype=np.float32)
    hand_coords = np.asarray(hand_coords, dtype=np.float32)
    B = gaze_coords.shape[0]
    assert B == N_CORES, f"expected batch {N_CORES}, got {B}"
    nc = _build()
    in_maps = [_in_map_for(gaze_coords, hand_coords, b) for b in range(B)]
    res = run_bass_kernel_spmd(nc, in_maps, list(range(N_CORES)),
                               trace=_trace, **trace_kwargs)
    out = np.stack(
        [res.results[i]["out"].reshape(S_DIM, C_DIM, H, W) for i in range(B)],
        axis=0,
    ).astype(np.float32)
    if _trace:
        return out, res
    return out
